# revision 8
# baseline (speedup 1.0000x reference)
"""Multi-head causal attention (B=2, T=2048, E=768, H=12, D=64) on 8 trn2 cores.

Sharding: core c handles batch b=c//4 and heads [3g, 3g+1, 3g+2] (g=c%4).
Each core computes its 3 heads' attention plus their partial contribution to the
final projection; the host sums the 4 partials per batch.

Per-core device program (all matmuls fp32r ~= tf32 precision):
  phase 1: qT/kT = (Wqk x^T + b)  four row-chunks [q0q1][k0k1][q2][k2]
           vT  = (Wv x^T + b)   [192, 2048] -> PE-transpose to v [s,d] tiles
                 with a ones column (for softmax denominators l = sum exp)
  phase 2: per head, per key block j: S^T = kT_j^T qT (K=64), +causal mask on
           diagonal block, P = exp(scale * S^T) (ACT, fused scale),
           [O^T; l] accumulated via matmul(lhsT=[v_j | 1], rhs=P).
           Normalize: recip = 1/l (DVE), partition-broadcast via K=1 matmul,
           elementwise multiply -> normalized O^T in SBUF.
  phase 3: out = sum_h O_h^T.T @ wf_h -> [2048, 768] partial, DMA out.

`repeat` unrolls the whole body N times in one NEFF; used by test.py to
measure per-body HW time as (t_N - t_1)/(N-1), cancelling dispatch overhead.
"""
import numpy as np

EMBED_DIM = 768
B = 2
T = 2048
N_CORES = 8
NT = T // 128           # 16 query/key tiles
SCALE = 1.0 / np.sqrt(64.0)
NEG = -1.0e9

_state = {}


def _build(repeat=1):
    import concourse.tile as tile
    from concourse import bacc, mybir
    from concourse.masks import make_identity

    F32 = mybir.dt.float32
    F32R = mybir.dt.float32r
    Exp = mybir.ActivationFunctionType.Exp
    ADD = mybir.AluOpType.add
    MULT = mybir.AluOpType.mult

    nc = bacc.Bacc("TRN2", target_bir_lowering=False, debug=False)

    xT_d = nc.dram_tensor("xT", [EMBED_DIM, T], F32R, kind="ExternalInput").ap()
    # columns ordered [q0 q1 | k0 k1 | q2 | k2]
    wqk_d = nc.dram_tensor("wqk", [EMBED_DIM, 384], F32R, kind="ExternalInput").ap()
    wv_d = nc.dram_tensor("wv", [EMBED_DIM, 192], F32R, kind="ExternalInput").ap()
    bqk_d = nc.dram_tensor("bqk", [384, 1], F32, kind="ExternalInput").ap()
    bv_d = nc.dram_tensor("bv", [192, 1], F32, kind="ExternalInput").ap()
    wf_d = nc.dram_tensor("wf", [192, EMBED_DIM], F32R, kind="ExternalInput").ap()
    mask_d = nc.dram_tensor("mask", [128, 128], F32, kind="ExternalInput").ap()
    out_d = nc.dram_tensor("out_p", [T, EMBED_DIM], F32, kind="ExternalOutput").ap()

    bounds = [(0, 128), (128, 256), (256, 320), (320, 384)]

    with tile.TileContext(nc) as tc:
        with tc.tile_pool(name="const", bufs=1) as const, \
             tc.tile_pool(name="persist", bufs=1) as persist:
            # ---- constants ----
            wqk_sb = const.tile([128, 6, 384], F32R)
            wv_sb = const.tile([128, 6, 192], F32R)
            nc.sync.dma_start(out=wqk_sb[:], in_=wqk_d.rearrange("(k p) c -> p k c", p=128))
            nc.sync.dma_start(out=wv_sb[:], in_=wv_d.rearrange("(k p) c -> p k c", p=128))
            bqk_sb = [const.tile([128, 1], F32, name=f"bqk{m}", tag=f"bqk{m}")
                      for m in range(2)]
            bqk_sb += [const.tile([64, 1], F32, name=f"bqk{m}", tag=f"bqk{m}")
                       for m in range(2, 4)]
            for m, (r0, r1) in enumerate(bounds):
                nc.sync.dma_start(out=bqk_sb[m][:], in_=bqk_d[r0:r1, :])
            bv_sb = [const.tile([128, 1], F32, name="bv0", tag="bv0"),
                     const.tile([64, 1], F32, name="bv1", tag="bv1")]
            nc.sync.dma_start(out=bv_sb[0][:], in_=bv_d[0:128, :])
            nc.sync.dma_start(out=bv_sb[1][:], in_=bv_d[128:192, :])
            wf_sb = [const.tile([64, EMBED_DIM], F32R, name=f"wf{h}", tag=f"wf{h}")
                     for h in range(3)]
            for h in range(3):
                nc.sync.dma_start(out=wf_sb[h][:], in_=wf_d[64 * h:64 * (h + 1), :])
            mask_sb = const.tile([128, 128], F32)
            nc.sync.dma_start(out=mask_sb[:], in_=mask_d[:])
            ident_f = const.tile([128, 128], F32)
            make_identity(nc, ident_f)
            ident_r = const.tile([128, 128], F32R)
            nc.vector.tensor_copy(out=ident_r[:], in_=ident_f[:])
            ones_f = const.tile([128, 64], F32)
            nc.vector.memset(ones_f[:], 1.0)
            ones_r = const.tile([128, 64], F32R)
            nc.vector.tensor_copy(out=ones_r[:], in_=ones_f[:])

            # ---- persistent activations ----
            qA = persist.tile([128, T], F32R)   # q0 @0:64, q1 @64:128
            kA = persist.tile([128, T], F32R)   # k0 @0:64, k1 @64:128
            qB = persist.tile([64, T], F32R)    # q2
            kB = persist.tile([64, T], F32R)    # k2
            v_all = persist.tile([128, NT, 3, 65], F32R)   # [v | 1] per head
            ot_sb = [persist.tile([64, T], F32R, name=f"ot{h}", tag=f"ot{h}")
                     for h in range(3)]          # normalized O^T per head

            for rep in range(repeat):
                _emit_body(nc, tc, rep, locals())

    nc.compile()
    return nc


def _emit_body(nc, tc, rep, env):
    """Emit one full forward pass (phases 1-3)."""
    from concourse import mybir

    F32 = mybir.dt.float32
    F32R = mybir.dt.float32r
    Exp = mybir.ActivationFunctionType.Exp
    ADD = mybir.AluOpType.add
    MULT = mybir.AluOpType.mult

    xT_d, out_d = env["xT_d"], env["out_d"]
    wqk_sb, wv_sb = env["wqk_sb"], env["wv_sb"]
    bqk_sb, bv_sb, wf_sb = env["bqk_sb"], env["bv_sb"], env["wf_sb"]
    mask_sb, ident_r, ones_f, ones_r = (env["mask_sb"], env["ident_r"],
                                        env["ones_f"], env["ones_r"])
    qA, kA, qB, kB = env["qA"], env["kA"], env["qB"], env["kB"]
    v_all, ot_sb = env["v_all"], env["ot_sb"]
    bounds = env["bounds"]

    # ---------------- phase 1: projections ----------------
    with tc.tile_pool(name=f"p1sb{rep}", bufs=1) as p1sb, \
         tc.tile_pool(name=f"p1ps{rep}", bufs=2, space="PSUM") as p1ps:
        # xT loaded as 6x4 column-chunk tiles so matmuls start early
        xT_t = [[p1sb.tile([128, 512], F32R, name=f"xT{rep}_{k}_{n}",
                           tag=f"xT{k}{n}") for n in range(4)] for k in range(6)]
        for n in range(4):
            for k in range(6):
                nc.sync.dma_start(
                    out=xT_t[k][n][:],
                    in_=xT_d[128 * k:128 * (k + 1), 512 * n:512 * (n + 1)])

        qk_dst = [qA, kA, qB, kB]
        for n in range(4):
            for m, (c0, c1) in enumerate(bounds):
                pm = c1 - c0
                ps = p1ps.tile([128, 512], F32, name=f"qkp{rep}_{m}{n}", tag="qkp")
                for k in range(6):
                    nc.tensor.matmul(
                        ps[:pm, :], lhsT=wqk_sb[:, k, c0:c1],
                        rhs=xT_t[k][n][:],
                        start=(k == 0), stop=(k == 5))
                nc.vector.tensor_scalar_add(
                    out=qk_dst[m][:pm, 512 * n:512 * (n + 1)],
                    in0=ps[:pm, :], scalar1=bqk_sb[m][:])

        # vT [192, 2048] then transpose into v_all
        vT_sb = [p1sb.tile([128, T], F32R, name=f"vT{rep}_0", tag="vT0"),
                 p1sb.tile([64, T], F32R, name=f"vT{rep}_1", tag="vT1")]
        for n in range(4):
            for m in range(2):
                pm = 128 if m == 0 else 64
                ps = p1ps.tile([128, 512], F32, name=f"vp{rep}_{m}{n}", tag="vp")
                for k in range(6):
                    nc.tensor.matmul(
                        ps[:pm, :], lhsT=wv_sb[:, k, 128 * m:128 * m + pm],
                        rhs=xT_t[k][n][:],
                        start=(k == 0), stop=(k == 5))
                nc.vector.tensor_scalar_add(
                    out=vT_sb[m][:pm, 512 * n:512 * (n + 1)],
                    in0=ps[:pm, :], scalar1=bv_sb[m][:pm, :])

        for i in range(NT):
            for h in range(3):
                m, off = divmod(64 * h, 128)
                tp = p1ps.tile([128, 64], F32R, name=f"tp{rep}_{i}{h}", tag="tp")
                nc.tensor.transpose(
                    tp[:], vT_sb[m][off:off + 64, 128 * i:128 * (i + 1)],
                    ident_r[off:off + 64, off:off + 64])
                nc.vector.tensor_copy(out=v_all[:, i, h, 0:64], in_=tp[:])
            for h in range(3):
                nc.vector.tensor_copy(out=v_all[:, i, h, 64:65],
                                      in_=ones_f[:, 0:1])

    # ---------------- phase 2: attention ----------------
    head_cfg = [(qA, kA, 0), (qA, kA, 64), (qB, kB, 0)]
    with tc.tile_pool(name=f"p2sb{rep}", bufs=2) as p2sb, \
         tc.tile_pool(name=f"p2ps{rep}", bufs=1, space="PSUM") as p2ps:
        for h in range(3):
            qT, kT, o = head_cfg[h]
            otl = p2ps.tile([128, T], F32, name=f"otl{rep}_{h}", tag="otl")
            for j in range(NT):
                g0 = (128 * j) // 512
                for g in range(g0, 4):
                    s0 = max(512 * g, 128 * j)
                    s1 = 512 * (g + 1)
                    ln = s1 - s0
                    st = p2ps.tile([128, 512], F32, name=f"st{rep}_{h}{j}{g}",
                                   tag=f"st{g}")
                    nc.tensor.matmul(
                        st[:, 0:ln],
                        lhsT=kT[o:o + 64, 128 * j:128 * (j + 1)],
                        rhs=qT[o:o + 64, s0:s1],
                        start=True, stop=True)
                    if g == g0:
                        nc.vector.tensor_tensor(
                            out=st[:, 0:128], in0=st[:, 0:128],
                            in1=mask_sb[:], op=ADD)
                    pt = p2sb.tile([128, 512], F32R, name=f"pt{rep}_{h}{j}{g}",
                                   tag=f"pt{g}")
                    nc.scalar.activation(out=pt[:, 0:ln], in_=st[:, 0:ln],
                                         func=Exp, scale=float(SCALE))
                    nc.tensor.matmul(
                        otl[0:65, s0:s1], lhsT=v_all[:, j, h, :],
                        rhs=pt[:, 0:ln],
                        start=(j == 0), stop=(j == min(4 * g + 3, NT - 1)))
            # normalize rows 0:64 by 1/l (row 64)
            recip = p2sb.tile([128, T], F32R, name=f"recip{rep}_{h}", tag="recip")
            with nc.allow_low_precision(reason="fp32r recip feeds bcast matmul"):
                nc.vector.reciprocal(out=recip[64:65, :], in_=otl[64:65, :])
            for g in range(4):
                bc = p2ps.tile([128, 512], F32, name=f"bc{rep}_{h}{g}", tag=f"st{g}")
                nc.tensor.matmul(bc[0:64, :],
                                 lhsT=ones_r[64:65, :],
                                 rhs=recip[64:65, 512 * g:512 * (g + 1)],
                                 start=True, stop=True)
                bcs = p2sb.tile([128, 512], F32, name=f"bcs{rep}_{h}{g}",
                                tag=f"bcs{g}")
                nc.vector.tensor_copy(out=bcs[0:64, :], in_=bc[0:64, :])
                nc.vector.tensor_tensor(
                    out=ot_sb[h][:, 512 * g:512 * (g + 1)],
                    in0=otl[0:64, 512 * g:512 * (g + 1)],
                    in1=bcs[0:64, :], op=MULT)

    # ---------------- phase 3: output projection ----------------
    with tc.tile_pool(name=f"p3sb{rep}", bufs=3) as p3sb, \
         tc.tile_pool(name=f"p3ps{rep}", bufs=2, space="PSUM") as p3ps:
        for i in range(NT):
            fp = p3ps.tile([128, EMBED_DIM], F32, name=f"fp{rep}_{i}", tag="fp")
            for (n0, n1) in [(0, 512), (512, 768)]:
                for h in range(3):
                    nc.tensor.matmul(fp[:, n0:n1],
                                     lhsT=ot_sb[h][:, 128 * i:128 * (i + 1)],
                                     rhs=wf_sb[h][:, n0:n1],
                                     start=(h == 0), stop=(h == 2))
            ob = p3sb.tile([128, EMBED_DIM], F32, name=f"ob{rep}_{i}", tag="ob")
            if i % 2 == 0:
                nc.scalar.copy(out=ob[:], in_=fp[:])
            else:
                nc.vector.tensor_copy(out=ob[:], in_=fp[:])
            nc.sync.dma_start(out=out_d[128 * i:128 * (i + 1), :], in_=ob[:])


def _prep_inputs(x, w_qkv, b_qkv, w_final):
    """Build the 8 per-core input maps from the full inputs."""
    x = np.asarray(x, dtype=np.float32)
    w_qkv = np.asarray(w_qkv, dtype=np.float32)
    b_qkv = np.asarray(b_qkv, dtype=np.float32)
    w_final = np.asarray(w_final, dtype=np.float32)
    E = EMBED_DIM

    mask = np.where(np.arange(128)[:, None] <= np.arange(128)[None, :], 0.0, NEG
                    ).astype(np.float32)
    in_maps = []
    for c in range(N_CORES):
        b = c // 4
        g = c % 4
        heads = [3 * g, 3 * g + 1, 3 * g + 2]
        hr = [np.arange(64 * h, 64 * h + 64) for h in heads]
        # [q0 q1 | k0 k1 | q2 | k2]
        rows_qk = np.concatenate([hr[0], hr[1], E + hr[0], E + hr[1], hr[2], E + hr[2]])
        rows_v = np.concatenate(hr) + 2 * E
        xT = np.ascontiguousarray(x[b].T)                       # [768, 2048]
        wqk = np.ascontiguousarray(w_qkv[rows_qk].T)            # [768, 384]
        wv = np.ascontiguousarray(w_qkv[rows_v].T)              # [768, 192]
        bqk = np.ascontiguousarray(b_qkv[rows_qk][:, None])
        bv = np.ascontiguousarray(b_qkv[rows_v][:, None])
        wf = np.ascontiguousarray(w_final[:, np.concatenate(hr)].T)  # [192, 768]
        in_maps.append({"xT": xT, "wqk": wqk, "wv": wv, "bqk": bqk, "bv": bv,
                        "wf": wf, "mask": mask})
    return in_maps


def kernel(x, w_qkv, b_qkv, w_final, _trace=False):
    from concourse.bass_utils import run_bass_kernel_spmd

    if "nc" not in _state:
        _state["nc"] = _build()
    nc = _state["nc"]

    in_maps = _prep_inputs(x, w_qkv, b_qkv, w_final)
    res = run_bass_kernel_spmd(nc, in_maps, list(range(N_CORES)), trace=_trace)
    _state["last_result"] = res

    out = np.empty((B, T, EMBED_DIM), dtype=np.float32)
    for b in range(B):
        acc = np.zeros((T, EMBED_DIM), dtype=np.float64)
        for g in range(4):
            acc += res.results[4 * b + g]["out_p"].astype(np.float64)
        out[b] = acc.astype(np.float32)
    return out
